# revision 34
# baseline (speedup 1.0000x reference)
"""ARAP energy kernel v6 — vertex-major edge stream, all math on device.

Edge neighbor coordinates (V_j, Vd_j; 6 x bf16 per edge) are laid out
vertex-major by the host ([128 = v%128, tile, slot, 6]) and streamed in with
plain dense DMA. The device applies the weights, forms the per-edge outer
products, and reduces per vertex with strided tensor_reduce straight into
the Gall layout the SVD phase consumes. No gather primitive is used on
device at all; every engine op is a dense vector op.
"""
import numpy as np
import concourse.bacc as bacc
import concourse.bass as bass
import concourse.tile as tile
from concourse import mybir
from concourse.bass_utils import run_bass_kernel_spmd
from contextlib import ExitStack

F32 = mybir.dt.float32
BF16 = mybir.dt.bfloat16
I32 = mybir.dt.int32
U8 = mybir.dt.uint8
AL = mybir.AluOpType
AF = mybir.ActivationFunctionType

N_CORES = 8
NV, K = 200000, 32
PART = 128
TILES = 196
NC_V = PART * TILES            # 25088 vertices per core
NPAD = N_CORES * NC_V          # 200704
T_CH = 14                      # tiles per chunk
NCH = TILES // T_CH            # 14 chunks
SLOT_CH = T_CH * K             # 448 slots per partition per chunk

GAMMA = float(3.0 + 2.0 * np.sqrt(2.0))
CPI8 = float(np.cos(np.pi / 8))
SPI8 = float(np.sin(np.pi / 8))
SWEEPS = 2

BF16_NP = mybir.dt.np(BF16)


def prep(V, V_def, nbrs, wgts):
    V = np.ascontiguousarray(V, np.float32)
    Vd = np.ascontiguousarray(V_def, np.float32)
    nbrs64 = np.ascontiguousarray(nbrs).astype(np.int64)
    wgts = np.ascontiguousarray(wgts, np.float32)

    Vp = np.zeros((NPAD, 3), np.float32); Vp[:NV] = V
    Vdp = np.zeros((NPAD, 3), np.float32); Vdp[:NV] = Vd
    nb = np.zeros((NPAD, K), np.int64); nb[:NV] = nbrs64
    w = np.zeros((NPAD, K), np.float32); w[:NV] = wgts

    # per-edge neighbor coordinates, vertex-major: vertex v = t*128 + p owns
    # slots [p, t, s]; padding slots have zero coords and zero weight
    nbz = np.where(w != 0.0, nb, 0)
    ecoord = np.empty((NPAD, K, 6), np.float32)
    ecoord[:, :, 0:3] = Vp[nbz]
    ecoord[:, :, 3:6] = Vdp[nbz]
    ecoord[w == 0.0] = 0.0

    in_maps = []
    for c in range(N_CORES):
        sl = slice(c * NC_V, (c + 1) * NC_V)
        ec = ecoord[sl].reshape(TILES, PART, K * 6).transpose(1, 0, 2)\
            .reshape(PART, TILES * K * 6).astype(BF16_NP)
        w6 = np.repeat(w[sl], 6, axis=1).reshape(TILES, PART, K * 6)\
            .transpose(1, 0, 2).reshape(PART, TILES * K * 6).astype(BF16_NP)
        own8 = np.zeros((NC_V, 8), np.float32)
        own8[:, 0:3] = Vp[sl]; own8[:, 4:7] = Vdp[sl]
        own8[:, 3] = w[sl].sum(1)
        own_c = own8.reshape(TILES, PART, 8).transpose(1, 0, 2)\
            .reshape(PART, TILES * 8)
        in_maps.append({
            "ecoord": np.ascontiguousarray(ec),
            "wrep6": np.ascontiguousarray(w6),
            "own8": np.ascontiguousarray(own_c),
        })
    return in_maps


class P:
    _ctr = [0]
    def __init__(self, nc, pool, eng):
        self.nc, self.pool, self.eng = nc, pool, eng
    def new(self, tag=None):
        self._ctr[0] += 1
        return self.pool.tile([PART, TILES], F32, tag=tag, name=f"{tag}_{self._ctr[0]}")
    def tt(self, out, a, b, op):
        self.eng.tensor_tensor(out=out, in0=a, in1=b, op=op); return out
    def ts(self, out, a, s1, op, s2=None, op2=None):
        if s2 is None:
            self.eng.tensor_scalar(out=out, in0=a, scalar1=float(s1), scalar2=None, op0=op)
        else:
            self.eng.tensor_scalar(out=out, in0=a, scalar1=float(s1), scalar2=float(s2), op0=op, op1=op2)
        return out
    def stt(self, out, a, s, b, op0, op1):
        self.eng.scalar_tensor_tensor(out=out, in0=a, scalar=float(s), in1=b, op0=op0, op1=op1); return out
    def sel(self, out, mask, t, f):
        self.eng.select(out=out, mask=mask, on_true=t, on_false=f); return out
    def act(self, S, out, a, func, bias=0.0, scale=1.0):
        S.activation(out=out, in_=a, func=func, bias=bias, scale=scale); return out
    def rsqrt(self, S, out, a, bias_ap):
        S.activation(out=out, in_=a, func=AF.Sqrt, bias=bias_ap)
        self.eng.reciprocal(out=out, in_=out); return out


def build_kernel(debug=False):
    nc = bacc.Bacc("TRN2", target_bir_lowering=False, debug=False, num_devices=N_CORES)
    ec_d = nc.dram_tensor("ecoord", [PART, TILES * K * 6], BF16, kind="ExternalInput").ap()
    w6_d = nc.dram_tensor("wrep6", [PART, TILES * K * 6], BF16, kind="ExternalInput").ap()
    own_d = nc.dram_tensor("own8", [PART, TILES * 8], F32, kind="ExternalInput").ap()
    e_out = nc.dram_tensor("e_out", [PART, TILES], F32, kind="ExternalOutput").ap()
    dbg = {}
    if debug:
        dbg["gall"] = nc.dram_tensor("dbg_gall", [PART, TILES * 16], F32, kind="ExternalOutput").ap()
        for nm in ("det", "ra", "cpl", "b00", "b11", "b22", "w0", "rs0"):
            dbg[nm] = nc.dram_tensor("dbg_" + nm, [PART, TILES], F32, kind="ExternalOutput").ap()

    CH6 = SLOT_CH * 6

    with tile.TileContext(nc) as tc, ExitStack() as ctx:
        persist = ctx.enter_context(tc.tile_pool(name="persist", bufs=1))
        gio = ctx.enter_context(tc.tile_pool(name="gio", bufs=2))
        tmp = ctx.enter_context(tc.tile_pool(name="tmp", bufs=1))

        Vv = nc.vector
        S = nc.scalar

        own_t = persist.tile([PART, TILES * 8], F32, name="own_t")
        nc.sync.dma_start(out=own_t[:], in_=own_d)
        Gall = persist.tile([PART, TILES * 16], F32, name="Gall")

        def tree_sum(eng, Xv, final_out=None):
            # Xv: [p, t, K, n] bf16 view; in-place halving sum over the slot
            # axis (packed last dim keeps DVE 2x/4x modes). The h==1 step
            # writes f32 into final_out [p, t, 1, n] if given.
            h = K // 2
            while h >= 1:
                in0 = Xv[:, :, 0:h, :]
                in1 = Xv[:, :, h:2 * h, :]
                out = in0 if not (h == 1 and final_out is not None) else final_out
                eng.tensor_tensor(out=out, in0=in0, in1=in1, op=AL.add)
                h //= 2

        for c in range(NCH):
            ec_t = gio.tile([PART, CH6], BF16, tag="ec", name=f"ec{c}")
            nc.sync.dma_start(out=ec_t[:], in_=ec_d[:, c * CH6:(c + 1) * CH6])
            w6_t = gio.tile([PART, CH6], BF16, tag="w6", name=f"w6{c}")
            nc.sync.dma_start(out=w6_t[:], in_=w6_d[:, c * CH6:(c + 1) * CH6])
            # Xw = (w*V_j, w*Vd_j) per slot
            Xw = gio.tile([PART, CH6], BF16, tag="Xw", name=f"Xw{c}")
            Vv.tensor_tensor(out=Xw[:], in0=ec_t[:], in1=w6_t[:], op=AL.mult)
            # P9[a,b] = (w*Vd_a) * V_b per slot
            P9 = gio.tile([PART, SLOT_CH * 9], BF16, tag="P9", name=f"P9{c}")
            Vv.tensor_tensor(
                out=P9[:].rearrange("p (m a b) -> p m a b", a=3, b=3),
                in0=Xw[:].rearrange("p (m e) -> p m e", e=6)[:, :, 3:6]
                    [:, :, :, None].to_broadcast([PART, SLOT_CH, 3, 3]),
                in1=ec_t[:].rearrange("p (m e) -> p m e", e=6)[:, :, 0:3]
                    [:, :, None, :].to_broadcast([PART, SLOT_CH, 3, 3]),
                op=AL.mult)
            # M6 = (w*V.V, w*Vd.Vd) componentwise, summed later into q
            M6 = gio.tile([PART, CH6], BF16, tag="M6", name=f"M6{c}")
            Vv.tensor_tensor(out=M6[:], in0=Xw[:], in1=ec_t[:], op=AL.mult)
            gsl = Gall[:, c * T_CH * 16:(c + 1) * T_CH * 16]\
                .rearrange("p (t f) -> p t f", f=16)
            tree_sum(Vv, P9[:].rearrange("p (t s n) -> p t s n", s=K, n=9),
                     final_out=gsl[:, :, 0:9].unsqueeze(2))
            tree_sum(Vv, Xw[:].rearrange("p (t s e) -> p t s e", s=K, e=6),
                     final_out=gsl[:, :, 9:15].unsqueeze(2))
            M6v = M6[:].rearrange("p (t s e) -> p t s e", s=K, e=6)
            tree_sum(Vv, M6v)
            Vv.tensor_reduce(
                out=gsl[:, :, 15:16],
                in_=M6v[:, :, 0, :],
                axis=mybir.AxisListType.X, op=AL.add)

        if debug:
            nc.sync.dma_start(out=dbg["gall"], in_=Gall[:])

        # ---------------- corrections: A, c ----------------
        p = P(nc, tmp, Vv)
        pg = P(nc, tmp, nc.gpsimd)
        gv = Gall[:].rearrange("p (t f) -> p t f", f=16)
        ownv = own_t[:].rearrange("p (t e) -> p t e", e=8)
        wt = ownv[:, :, 3]

        t1 = p.new("t1"); t2_ = p.new("t2"); t3 = p.new("t3")
        g1 = pg.new("g1"); g2 = pg.new("g2"); g3 = pg.new("g3")
        # m2t[b] = m2[b] - wt*V_n[b] folds the wt*Vd(x)V term into A
        m2t = []
        for b in range(3):
            mb = persist.tile([PART, TILES], F32, tag=f"m2t{b}", name=f"m2t{b}")
            p.tt(mb[:], wt, ownv[:, :, b], AL.mult)
            p.tt(mb[:], gv[:, :, 9 + b], mb[:], AL.subtract)
            m2t.append(mb)
        A = {}
        for a in range(3):
            for b in range(3):
                ap_ = persist.tile([PART, TILES], F32, tag=f"A{a}{b}", name=f"A{a}{b}")
                # A = M1 - Vd_n[a]*m2t[b] - m3[a]*V_n[b]
                p.tt(t1[:], ownv[:, :, 4 + a], m2t[b][:], AL.mult)
                p.tt(t2_[:], gv[:, :, 12 + a], ownv[:, :, b], AL.mult)
                p.tt(ap_[:], gv[:, :, 3 * a + b], t1[:], AL.subtract)
                p.tt(ap_[:], ap_[:], t2_[:], AL.subtract)
                A[(a, b)] = ap_
        cpl = persist.tile([PART, TILES], F32, name="cpl")
        # c = q - 2<V_n, m2> - 2<Vd_n, m3> + wt*(|V_n|^2+|Vd_n|^2)  (on gpsimd)
        pg.tt(g1[:], ownv[:, :, 0], gv[:, :, 9], AL.mult)
        for b in (1, 2):
            pg.tt(g2[:], ownv[:, :, b], gv[:, :, 9 + b], AL.mult)
            pg.tt(g1[:], g1[:], g2[:], AL.add)
        for a in (0, 1, 2):
            pg.tt(g2[:], ownv[:, :, 4 + a], gv[:, :, 12 + a], AL.mult)
            pg.tt(g1[:], g1[:], g2[:], AL.add)
        pg.tt(g3[:], ownv[:, :, 0], ownv[:, :, 0], AL.mult)
        for e in (1, 2, 4, 5, 6):
            pg.tt(g2[:], ownv[:, :, e], ownv[:, :, e], AL.mult)
            pg.tt(g3[:], g3[:], g2[:], AL.add)
        pg.tt(g3[:], wt, g3[:], AL.mult)
        p.stt(cpl[:], g1[:], -2.0, g3[:], AL.mult, AL.add)
        p.tt(cpl[:], cpl[:], gv[:, :, 15], AL.add)

        # ---------------- Jacobi SVD -> R -> E ----------------
        Bm = {}
        for i in range(3):
            for j in range(i, 3):
                bp = persist.tile([PART, TILES], F32, tag=f"B{i}{j}", name=f"B{i}{j}")
                p.tt(t1[:], A[(0, i)][:], A[(0, j)][:], AL.mult)
                p.tt(t2_[:], A[(1, i)][:], A[(1, j)][:], AL.mult)
                p.tt(t1[:], t1[:], t2_[:], AL.add)
                p.tt(t2_[:], A[(2, i)][:], A[(2, j)][:], AL.mult)
                p.tt(bp[:], t1[:], t2_[:], AL.add)
                Bm[(i, j)] = bp
        Vm = {}
        for i in range(3):
            for j in range(3):
                vp = persist.tile([PART, TILES], F32, tag=f"V{i}{j}", name=f"Vm{i}{j}")
                nc.gpsimd.memset(vp[:], 1.0 if i == j else 0.0)
                Vm[(i, j)] = vp
        cpi8 = persist.tile([PART, TILES], F32, tag="cpi8", name="cpi8")
        biasc = persist.tile([PART, 1], F32, tag="biasc", name="biasc")
        Vv.memset(biasc[:], 1e-30)
        spi8 = persist.tile([PART, TILES], F32, tag="spi8", name="spi8")
        Vv.memset(cpi8[:], CPI8)
        Vv.memset(spi8[:], SPI8)

        def b_at(i, j):
            return Bm[(min(i, j), max(i, j))]

        ROTS = [(0, 1), (0, 2), (1, 2)] * SWEEPS + [(0, 1), (0, 2)]
        if True:
            for sweep, (pp, qq) in enumerate(ROTS):
                bpp = b_at(pp, pp); bqq = b_at(qq, qq); bpq = b_at(pp, qq)
                ch_ = p.new("ch"); sh = p.new("sh")
                p.tt(ch_[:], bpp[:], bqq[:], AL.subtract)
                p.ts(sh[:], bpq[:], 0.5, AL.mult)
                ch2 = p.new("ch2"); sh2 = p.new("sh2")
                p.tt(ch2[:], ch_[:], ch_[:], AL.mult)
                p.tt(sh2[:], sh[:], sh[:], AL.mult)
                mask = tmp.tile([PART, TILES], U8, tag="masku8", name=f"m_{sweep}_{pp}{qq}")
                p.stt(mask[:], sh2[:], GAMMA, ch2[:], AL.mult, AL.is_lt)
                den = p.new("den")
                p.tt(den[:], ch2[:], sh2[:], AL.add)
                om = p.new("om")
                p.rsqrt(S, om[:], den[:], biasc[:])
                cht = p.new("cht"); sht = p.new("sht")
                p.tt(cht[:], om[:], ch_[:], AL.mult)
                p.tt(sht[:], om[:], sh[:], AL.mult)
                p.sel(ch_[:], mask[:], cht[:], cpi8[:])
                p.sel(sh[:], mask[:], sht[:], spi8[:])
                c = p.new("c"); s = p.new("s")
                p.tt(ch2[:], ch_[:], ch_[:], AL.mult)
                p.tt(sh2[:], sh[:], sh[:], AL.mult)
                p.tt(c[:], ch2[:], sh2[:], AL.subtract)
                p.stt(s[:], ch_[:], 2.0, sh[:], AL.mult, AL.mult)
                c2 = p.new("c2"); s2 = p.new("s2"); cs = p.new("cs")
                p.tt(c2[:], c[:], c[:], AL.mult)
                p.tt(s2[:], s[:], s[:], AL.mult)
                p.tt(cs[:], c[:], s[:], AL.mult)
                m1 = p.new("m1"); m2 = p.new("m2"); m3 = p.new("m3")
                m4 = p.new("m4"); m5 = p.new("m5")
                p.tt(m1[:], c2[:], bpp[:], AL.mult)
                p.tt(m2[:], cs[:], bpq[:], AL.mult)
                p.tt(m3[:], s2[:], bqq[:], AL.mult)
                p.tt(m4[:], s2[:], bpp[:], AL.mult)
                p.tt(m5[:], c2[:], bqq[:], AL.mult)
                dq = p.new("dq")
                p.tt(dq[:], bqq[:], bpp[:], AL.subtract)
                p.tt(dq[:], cs[:], dq[:], AL.mult)
                c2s2 = p.new("c2s2")
                p.tt(c2s2[:], c2[:], s2[:], AL.subtract)
                p.tt(t1[:], c2s2[:], bpq[:], AL.mult)
                p.tt(bpq[:], dq[:], t1[:], AL.add)
                p.stt(t1[:], m2[:], 2.0, m1[:], AL.mult, AL.add)
                p.tt(bpp[:], t1[:], m3[:], AL.add)
                p.stt(t2_[:], m2[:], -2.0, m4[:], AL.mult, AL.add)
                p.tt(bqq[:], t2_[:], m5[:], AL.add)
                rr = 3 - pp - qq
                x = b_at(pp, rr); y = b_at(qq, rr)
                xn = p.new("xn")
                p.tt(t1[:], c[:], x[:], AL.mult)
                p.tt(t2_[:], s[:], y[:], AL.mult)
                p.tt(t3[:], c[:], y[:], AL.mult)
                p.tt(xn[:], s[:], x[:], AL.mult)
                p.tt(x[:], t1[:], t2_[:], AL.add)
                p.tt(y[:], t3[:], xn[:], AL.subtract)
                g4 = pg.new("g4")
                for i in range(3):
                    vip = Vm[(i, pp)]; viq = Vm[(i, qq)]
                    pg.tt(g1[:], c[:], vip[:], AL.mult)
                    pg.tt(g2[:], s[:], viq[:], AL.mult)
                    pg.tt(g3[:], c[:], viq[:], AL.mult)
                    pg.tt(g4[:], s[:], vip[:], AL.mult)
                    pg.tt(vip[:], g1[:], g2[:], AL.add)
                    pg.tt(viq[:], g3[:], g4[:], AL.subtract)

        # at convergence the rotating Bm's diagonal holds the eigenvalues
        # sigma_j^2 directly — clamp at 0 (roundoff can leave tiny negatives
        # on rank-deficient covariances, which would blow up rsig * sig2)
        sig2 = []
        for j in range(3):
            scj = persist.tile([PART, TILES], F32, tag=f"s2c{j}", name=f"s2c{j}")
            p.ts(scj[:], b_at(j, j)[:], 0.0, AL.max)
            sig2.append(scj)
        det = persist.tile([PART, TILES], F32, tag="det", name="det")
        pg.tt(g1[:], A[(1, 1)][:], A[(2, 2)][:], AL.mult)
        pg.tt(g2[:], A[(1, 2)][:], A[(2, 1)][:], AL.mult)
        pg.tt(g1[:], g1[:], g2[:], AL.subtract)
        pg.tt(det[:], A[(0, 0)][:], g1[:], AL.mult)
        pg.tt(g1[:], A[(1, 0)][:], A[(2, 2)][:], AL.mult)
        pg.tt(g2[:], A[(1, 2)][:], A[(2, 0)][:], AL.mult)
        pg.tt(g1[:], g1[:], g2[:], AL.subtract)
        pg.tt(g1[:], A[(0, 1)][:], g1[:], AL.mult)
        pg.tt(det[:], det[:], g1[:], AL.subtract)
        pg.tt(g1[:], A[(1, 0)][:], A[(2, 1)][:], AL.mult)
        pg.tt(g2[:], A[(1, 1)][:], A[(2, 0)][:], AL.mult)
        pg.tt(g1[:], g1[:], g2[:], AL.subtract)
        pg.tt(g1[:], A[(0, 2)][:], g1[:], AL.mult)
        pg.tt(det[:], det[:], g1[:], AL.add)
        sgn = p.new("sgn")
        p.ts(t1[:], det[:], 0.0, AL.is_lt)
        p.ts(sgn[:], t1[:], -2.0, AL.mult, 1.0, AL.add)
        f0 = p.new("f0"); f1 = p.new("f1"); f2 = p.new("f2")
        p.tt(t1[:], sig2[0][:], sig2[1][:], AL.is_le)
        p.tt(t2_[:], sig2[0][:], sig2[2][:], AL.is_le)
        p.tt(f0[:], t1[:], t2_[:], AL.mult)
        p.ts(t3[:], f0[:], -1.0, AL.mult, 1.0, AL.add)
        p.tt(t1[:], sig2[1][:], sig2[2][:], AL.is_le)
        p.tt(f1[:], t3[:], t1[:], AL.mult)
        p.tt(t3[:], f0[:], f1[:], AL.add)
        p.ts(f2[:], t3[:], -1.0, AL.mult, 1.0, AL.add)
        sgn1 = p.new("sgn1")
        p.ts(sgn1[:], sgn[:], -1.0, AL.add)
        rsig = []
        for j, fj in enumerate((f0, f1, f2)):
            rp = p.new(f"rsig{j}")
            p.tt(t1[:], fj[:], sgn1[:], AL.mult)
            p.ts(t1[:], t1[:], 1.0, AL.add)
            p.rsqrt(S, t2_[:], sig2[j][:], biasc[:])
            p.tt(rp[:], t1[:], t2_[:], AL.mult)
            rsig.append(rp)
        # ra = tr(A^T R), R = U.Vm (reference's R = U.V convention). Using
        # A = U.diag(sig).Vm^T at convergence:
        # ra = sum_j rsig_j * sig2_j * (Vm.Vm)[j,j]
        q01 = p.new("q01"); q02 = p.new("q02"); q12 = p.new("q12")
        p.tt(q01[:], Vm[(0, 1)][:], Vm[(1, 0)][:], AL.mult)
        p.tt(q02[:], Vm[(0, 2)][:], Vm[(2, 0)][:], AL.mult)
        p.tt(q12[:], Vm[(1, 2)][:], Vm[(2, 1)][:], AL.mult)
        ra = p.new("ra")
        first = True
        for j, (da, qa, qb) in enumerate((((0, 0), q01, q02),
                                          ((1, 1), q01, q12),
                                          ((2, 2), q02, q12))):
            wj = p.new(f"w{j}")
            p.tt(wj[:], Vm[da][:], Vm[da][:], AL.mult)
            p.tt(wj[:], wj[:], qa[:], AL.add)
            p.tt(wj[:], wj[:], qb[:], AL.add)
            p.tt(t1[:], rsig[j][:], sig2[j][:], AL.mult)
            if first:
                p.tt(ra[:], t1[:], wj[:], AL.mult)
                first = False
            else:
                p.tt(t1[:], t1[:], wj[:], AL.mult)
                p.tt(ra[:], ra[:], t1[:], AL.add)
        epl = p.new("epl")
        p.stt(epl[:], ra[:], -2.0, cpl[:], AL.mult, AL.add)
        nc.sync.dma_start(out=e_out, in_=epl[:])
        if debug:
            nc.sync.dma_start(out=dbg["det"], in_=det[:])
            nc.sync.dma_start(out=dbg["ra"], in_=ra[:])
            nc.sync.dma_start(out=dbg["cpl"], in_=cpl[:])
            nc.sync.dma_start(out=dbg["b00"], in_=b_at(0, 0)[:])
            nc.sync.dma_start(out=dbg["b11"], in_=b_at(1, 1)[:])
            nc.sync.dma_start(out=dbg["b22"], in_=b_at(2, 2)[:])
            nc.sync.dma_start(out=dbg["w0"], in_=wj[:])
            nc.sync.dma_start(out=dbg["rs0"], in_=rsig[0][:])

    nc.compile()
    return nc


_cache = {}

def kernel(V, V_def, nbrs, wgts, _trace=False):
    """Full-input entry point: shards internally across 8 NeuronCores."""
    V = np.asarray(V, np.float32)
    V_def = np.asarray(V_def, np.float32)
    wgts = np.asarray(wgts, np.float32)
    nbrs = np.asarray(nbrs)
    if "nc" not in _cache:
        _cache["nc"] = build_kernel(debug=False)
    nc = _cache["nc"]
    in_maps = prep(V, V_def, nbrs, wgts)
    res = run_bass_kernel_spmd(nc, in_maps, list(range(N_CORES)), trace=_trace)
    total = 0.0
    for c in range(N_CORES):
        total += float(res.results[c]["e_out"].astype(np.float64).sum())
    out = np.float32(total / NV)
    _cache["last_res"] = res
    return out
